# revision 22
# baseline (speedup 1.0000x reference)
"""Trainium2 Bass kernel for an AttentionBlock (GroupNorm + single-head
full N^2 attention + output projection + residual), data-parallel over
batch: 8 samples on 8 NeuronCores, no collectives.

Shapes (hardcoded): x [8, 256, 64, 64]; weights [256, 256]; biases [256].
Per core: one batch sample, x viewed as [C=256, N=4096] channel-major.

Per-core pipeline (fp8 DoubleRow matmuls: 2 k-tiles of 128 contracted per
instruction -> full C=256 contraction per matmul at double fp8 rate):
  1. GroupNorm (8 groups) in C-major layout: per-partition bn_stats,
     cross-partition group reduction via tiny constant matmuls, applied as
     per-partition scale/bias. Rounded tokens t8 (fp8e4) feed all matmuls;
     the fp32 residual out = x*s + proj' is applied in the epilogue via
     scalar_tensor_tensor, and the bias b' = b_gn + bp + Wp bv rides the
     V3 psum as a ones x b'-row outer product.
  2. Wq/Wk fold: scores = t A t^T with A = Wq^T Wk (computed on-chip,
     stored dual fp8 hi+lo). The bq bias becomes a +Wk^T bq column on the
     q2 projection (all other bias terms are per-query constants that
     cancel in softmax). The output projection folds into V:
     v3 = t (Wp Wv)^T (dual fp8), with appended ones columns so PV also
     produces softmax denominators.
  3. Attention over 512-query chunks, transposed: S^T = t8^T q8 with keys
     on partitions. Scores for a key-block PAIR live in one 2-bank psum
     tile so each exp instruction covers 1024 elems/partition. exp is
     split: ~9/16 pairs on ACT (native Exp -> fp8), ~7/16 via DVE
     Schraudolph (int32 = s*EA + EB == fp32 bits of exp(s*scale - SHIFT))
     with the int32 -> fp8 convert on GPSIMD (no PSUM port, so it only
     gets SBUF-side work). The global SHIFT keeps exp within fp8e4 range
     (max 240) and cancels in the normalization.
  4. PV accumulates [proj | colsum] in PSUM over 16 key-block pairs
     (DoubleRow). Epilogue per 128-query block: normalize by 1/colsum
     (ACT copy with per-partition scale), TensorE-transpose back to
     C-major, one scalar_tensor_tensor out = x*s + proj', DMA out.
"""

import numpy as np

import concourse.bacc as bacc
import concourse.mybir as mybir
import concourse.tile as tile
from concourse import bass_utils

F32 = mybir.dt.float32
F32R = mybir.dt.float32r
BF16 = mybir.dt.bfloat16
FP8 = mybir.dt.float8e4
I32 = mybir.dt.int32
AF = mybir.ActivationFunctionType
OP = mybir.AluOpType
DR = mybir.MatmulPerfMode.DoubleRow

B = 8
C = 256
H = 64
W = 64
N = H * W  # 4096 tokens
G = 8  # groups
GS = C // G  # 32 channels per group
P = 128
CB = C // P  # 2 channel blocks
EPS = 1e-5
NCHUNK = 256  # query chunk (matmul moving free dim)
NJ = N // NCHUNK  # 16
MB = N // P  # 32 key blocks
NPAIR = MB // 2  # 16 key-block pairs
JJ = NCHUNK // P  # 2 query sub-blocks per chunk
SCALE = C ** (-0.5)
SHIFT = 3.5  # exp(s*SCALE - SHIFT): keeps fp8e4 range, cancels in softmax

# Schraudolph exp-as-int-bits constants: exp(y) ~= bitcast_f32(int32(y*EA + EB))
LOG2E = 1.4426950408889634
EA = float(SCALE * (1 << 23) * LOG2E)
EB = float((1 << 23) * (127.0 - SHIFT * LOG2E - 0.0450))

# Engine split for the 16 exp pair-tiles per chunk: "A" = ACT native exp,
# "D" = DVE Schraudolph pass1 + GPSIMD fp8-convert pass2.
PAIR_SCHED = ["A", "D"] * 8

_CACHE: dict = {}


def build_nc(att_reps=1, exp_mode="mix"):
    """exp_mode: "mix" (PAIR_SCHED), "act", "dve" (timing calibration),
    "none" (skip exp: PV reads a constant tile; output garbage)."""
    nc = bacc.Bacc(
        "TRN2",
        target_bir_lowering=False,
        debug=False,
        enable_asserts=False,
        num_devices=B,
    )

    x_d = nc.dram_tensor("x", [C, N], F32, kind="ExternalInput")
    gamma_d = nc.dram_tensor("gamma", [C], F32, kind="ExternalInput")
    beta_d = nc.dram_tensor("beta", [C], F32, kind="ExternalInput")
    w_d = {}
    b_d = {}
    for nm in ("q", "k", "v", "p"):
        w_d[nm] = nc.dram_tensor(f"W{nm}", [C, C], F32, kind="ExternalInput")
        b_d[nm] = nc.dram_tensor(f"b{nm}", [C], F32, kind="ExternalInput")
    out_d = nc.dram_tensor("out", [C, N], F32, kind="ExternalOutput")

    ident_d = nc.inline_tensor(np.eye(P, dtype=np.float32), name="ident")
    # Group-sum selector: [P, G/CB] with 1/GS entries -> group means directly.
    gsum_np = np.zeros((P, G // CB), np.float32)
    for p in range(P):
        gsum_np[p, p // GS] = 1.0 / GS
    gsum_d = nc.inline_tensor(gsum_np, name="gsum")
    # Group-broadcast selector: [G/CB, P] with 1s.
    gbc_np = np.zeros((G // CB, P), np.float32)
    for p in range(P):
        gbc_np[p // GS, p] = 1.0
    gbc_d = nc.inline_tensor(gbc_np, name="gbc")

    from contextlib import ExitStack

    with tile.TileContext(nc) as tc:
        with ExitStack() as ctx:
            _build_tile(
                ctx, tc, x_d, gamma_d, beta_d, w_d, b_d, out_d, ident_d, gsum_d,
                gbc_d, att_reps, exp_mode,
            )
    nc.compile()
    return nc


def _build_tile(ctx, tc, x_d, gamma_d, beta_d, w_d, b_d, out_d, ident_d, gsum_d, gbc_d, att_reps=1, exp_mode="mix"):
    nc = tc.nc

    persist = ctx.enter_context(tc.tile_pool(name="persist", bufs=1))
    staging = ctx.enter_context(tc.tile_pool(name="staging", bufs=4))
    sexp = ctx.enter_context(tc.tile_pool(name="sexp", bufs=6))
    si32 = ctx.enter_context(tc.tile_pool(name="si32", bufs=4))
    sout = ctx.enter_context(tc.tile_pool(name="sout", bufs=4))
    stmp = ctx.enter_context(tc.tile_pool(name="stmp", bufs=6))
    # ps_pair: [P, 2, NCHUNK] f32 tiles (1 psum bank each, bufs=4 -> 4 banks)
    ps_pair = ctx.enter_context(tc.tile_pool(name="ps_pair", bufs=4, space="PSUM"))
    # ps_pv: PV accumulators (1 bank each, JJ=2 alive per chunk)
    ps_pv = ctx.enter_context(tc.tile_pool(name="ps_pv", bufs=2, space="PSUM"))
    # ps_tp: epilogue transposes (decoupled from PV accumulator rotation)
    ps_tp = ctx.enter_context(tc.tile_pool(name="ps_tp", bufs=2, space="PSUM"))

    t_cm = persist.tile([P, CB, N], F32, tag="t_cm")  # raw x, C-major
    t8 = persist.tile([P, CB, N], FP8, tag="t8")  # groupnormed tokens, fp8
    NSUB = N // 512  # bn_stats free-dim limit

    # ---- x load first (3 DMA queues), then constants/weights ----
    for s_ in range(NSUB):
        sl = slice(s_ * 512, (s_ + 1) * 512)
        if s_ < 4:
            nc.sync.dma_start(out=t_cm[:, 0, sl], in_=x_d[0:P, sl])
        else:
            nc.scalar.dma_start(out=t_cm[:, 0, sl], in_=x_d[0:P, sl])
        nc.gpsimd.dma_start(out=t_cm[:, 1, sl], in_=x_d[P : 2 * P, sl])

    # small constants split across SP / ACT queues behind the x slices
    gsum = persist.tile([P, G // CB], F32, tag="gsum")
    nc.scalar.dma_start(out=gsum, in_=gsum_d[:, :])
    gbc = persist.tile([G // CB, P], F32, tag="gbc")
    nc.scalar.dma_start(out=gbc, in_=gbc_d[:, :])

    def col_tile(dram_vec, tag, eng):
        t = persist.tile([P, CB], F32, tag=tag)
        eng.dma_start(out=t, in_=dram_vec[:].rearrange("(b p) -> p b", p=P))
        return t

    gamma_col = col_tile(gamma_d, "gamma_col", nc.scalar)
    beta_col = col_tile(beta_d, "beta_col", nc.scalar)
    ident = persist.tile([P, P], F32, tag="ident")
    nc.sync.dma_start(out=ident, in_=ident_d[:, :])
    bq_col = col_tile(b_d["q"], "bq_col", nc.sync)
    bv_col = col_tile(b_d["v"], "bv_col", nc.sync)
    bp_col = col_tile(b_d["p"], "bp_col", nc.sync)

    # staged natural-layout weights [P, CB, C] (row r = b*128+p on partition p)
    w_stage = {}
    for nm, eng in (("q", nc.sync), ("k", nc.sync), ("v", nc.scalar), ("p", nc.scalar)):
        w_sb = staging.tile([P, CB, C], F32, tag="w_stage", name=f"w_sb_{nm}")
        eng.dma_start(out=w_sb, in_=w_d[nm][:, :].rearrange("(b p) i -> p b i", p=P))
        w_stage[nm] = w_sb

    # ---- A = Wq^T Wk  [c, c'] as dual fp8 (hi + residual lo) ----
    a8 = persist.tile([P, CB, C], FP8, tag="a8")
    a8l = persist.tile([P, CB, C], FP8, tag="a8l")
    for cb in range(CB):
        aps = ps_pair.tile([P, C], F32, tag="ps_pair", name=f"aps_{cb}")
        for mb in range(CB):
            nc.tensor.matmul(
                aps,
                lhsT=w_stage["q"][:, mb, cb * P : (cb + 1) * P],
                rhs=w_stage["k"][:, mb, :],
                start=(mb == 0),
                stop=(mb == CB - 1),
            )
        nc.scalar.copy(out=a8[:, cb, :], in_=aps)
        nc.vector.tensor_tensor(
            out=a8l[:, cb, :], in0=aps, in1=a8[:, cb, :], op=OP.subtract
        )

    # ---- WpT via TensorE transposes: [P(m), CB(mb), C(c')] f32 ----
    wpT = persist.tile([P, CB, C], F32, tag="wpT")
    for b1 in range(CB):  # c' block (rows of Wp)
        for b2 in range(CB):  # m block
            tp = ps_pair.tile([P, P], F32, tag="ps_pair")
            nc.tensor.transpose(tp, w_stage["p"][:, b1, b2 * P : (b2 + 1) * P], ident)
            nc.scalar.copy(out=wpT[:, b2, b1 * P : (b1 + 1) * P], in_=tp)

    # ---- wvp8 = (Wp Wv)^T = Wv^T Wp^T  dual fp8 (hi + residual lo) ----
    wvp8 = persist.tile([P, CB, C], FP8, tag="wvp8")
    wvp8l = persist.tile([P, CB, C], FP8, tag="wvp8l")
    for ci_b in range(CB):
        pvp = ps_pair.tile([P, C], F32, tag="ps_pair", name=f"pvp_{ci_b}")
        for cm_b in range(CB):
            nc.tensor.matmul(
                pvp,
                lhsT=w_stage["v"][:, cm_b, ci_b * P : (ci_b + 1) * P],
                rhs=wpT[:, cm_b, :],
                start=(cm_b == 0),
                stop=(cm_b == CB - 1),
            )
        nc.scalar.copy(out=wvp8[:, ci_b, :], in_=pvp)
        nc.vector.tensor_tensor(
            out=wvp8l[:, ci_b, :], in0=pvp, in1=wvp8[:, ci_b, :], op=OP.subtract
        )

    # ---- w_col = Wk^T bq (q2 bias column), bv2_col = Wp bv ----
    w_col = persist.tile([P, CB], F32, tag="w_col")
    bv2_col = persist.tile([P, CB], F32, tag="bv2_col")
    for cb in range(CB):
        wps = ps_pair.tile([P, 1], F32, tag="ps_pair", name=f"wps_{cb}")
        for mb in range(CB):
            nc.tensor.matmul(
                wps,
                lhsT=w_stage["k"][:, mb, cb * P : (cb + 1) * P],
                rhs=bq_col[:, mb : mb + 1],
                start=(mb == 0),
                stop=(mb == CB - 1),
            )
        nc.vector.tensor_copy(out=w_col[:, cb : cb + 1], in_=wps)
        vps = ps_pair.tile([P, 1], F32, tag="ps_pair", name=f"vps_{cb}")
        for mb in range(CB):
            nc.tensor.matmul(
                vps,
                lhsT=wpT[:, mb, cb * P : (cb + 1) * P],
                rhs=bv_col[:, mb : mb + 1],
                start=(mb == 0),
                stop=(mb == CB - 1),
            )
        nc.vector.tensor_copy(out=bv2_col[:, cb : cb + 1], in_=vps)

    # ---- GroupNorm stats -> per-channel scale s_col, bias b_col ----
    gn_cols = []
    for cb in range(CB):
        xt = t_cm[:, cb, :]
        stats = stmp.tile([P, NSUB, 6], F32, tag="gn_stats")
        for s in range(NSUB):
            nc.vector.bn_stats(out=stats[:, s, :], in_=xt[:, s * 512 : (s + 1) * 512])
        mv = stmp.tile([P, 2], F32, tag="gn_mv")
        nc.vector.bn_aggr(out=mv, in_=stats)
        # stats2 = (mean_p, E[x^2]_p)
        stats2 = stmp.tile([P, 2], F32, tag="gn_stats2")
        nc.vector.tensor_copy(out=stats2[:, 0:1], in_=mv[:, 0:1])
        nc.vector.tensor_tensor(
            out=stats2[:, 1:2], in0=mv[:, 0:1], in1=mv[:, 0:1], op=OP.mult
        )
        nc.vector.tensor_add(out=stats2[:, 1:2], in0=stats2[:, 1:2], in1=mv[:, 1:2])
        # group reduce: [G/CB, 2] = gsum.T @ stats2  (means already /GS)
        gps = ps_pair.tile([G // CB, 2], F32, tag="ps_pair", name=f"gps_{cb}")
        nc.tensor.matmul(gps, lhsT=gsum, rhs=stats2, start=True, stop=True)
        # rstd_g = 1/sqrt(E2_g - mean_g^2 + eps)
        gsb = stmp.tile([G // CB, 2], F32, tag="gn_gsb")
        nc.vector.tensor_copy(out=gsb, in_=gps)
        gpack = stmp.tile([G // CB, 2], F32, tag="gn_gpack")
        nc.vector.tensor_copy(out=gpack[:, 0:1], in_=gsb[:, 0:1])
        gvar = stmp.tile([G // CB, 1], F32, tag="gn_gvar")
        nc.vector.tensor_tensor(
            out=gvar, in0=gsb[:, 0:1], in1=gsb[:, 0:1], op=OP.mult
        )
        nc.vector.tensor_tensor(
            out=gvar, in0=gsb[:, 1:2], in1=gvar, op=OP.subtract
        )
        eps_t = stmp.tile([G // CB, 1], F32, tag="gn_eps")
        nc.vector.memset(eps_t, EPS)
        nc.scalar.activation(out=gvar, in_=gvar, func=AF.Sqrt, bias=eps_t)
        nc.vector.reciprocal(out=gpack[:, 1:2], in_=gvar)
        # broadcast to channels: [P, 2] = gbc.T @ gpack
        bps = ps_pair.tile([P, 2], F32, tag="ps_pair", name=f"bps_{cb}")
        nc.tensor.matmul(bps, lhsT=gbc, rhs=gpack, start=True, stop=True)
        # s_col = rstd_c * gamma_c ; b_col = beta_c - mean_c * s_col
        s_col = stmp.tile([P, 1], F32, tag="gn_scol")
        nc.vector.tensor_tensor(
            out=s_col, in0=bps[:, 1:2], in1=gamma_col[:, cb : cb + 1], op=OP.mult
        )
        b_col = stmp.tile([P, 1], F32, tag="gn_bcol")
        nc.vector.tensor_tensor(out=b_col, in0=bps[:, 0:1], in1=s_col, op=OP.mult)
        nc.vector.tensor_tensor(
            out=b_col, in0=beta_col[:, cb : cb + 1], in1=b_col, op=OP.subtract
        )
        gn_cols.append((s_col, b_col))

    # ---- b' = b_gn + bp + Wp bv as a bf16 row [1, C] (for V3 psum fold) ----
    bsum_col = persist.tile([P, CB], F32, tag="bsum_col")
    for cb in range(CB):
        _, b_col = gn_cols[cb]
        nc.vector.tensor_add(
            out=bsum_col[:, cb : cb + 1], in0=b_col, in1=bp_col[:, cb : cb + 1]
        )
        nc.vector.tensor_add(
            out=bsum_col[:, cb : cb + 1],
            in0=bsum_col[:, cb : cb + 1],
            in1=bv2_col[:, cb : cb + 1],
        )
    bsum_bf = persist.tile([P, CB], BF16, tag="bsum_bf")
    nc.vector.tensor_copy(out=bsum_bf, in_=bsum_col)
    b_row = persist.tile([1, C], BF16, tag="b_row")
    for cb in range(CB):
        nc.sync.dma_start(
            out=b_row[0:1, cb * P : (cb + 1) * P], in_=bsum_bf[:, cb : cb + 1]
        )
    ones_bf = persist.tile([1, P], BF16, tag="ones_bf")
    nc.vector.memset(ones_bf, 1.0)
    nshift_col = persist.tile([P, 1], F32, tag="nshift_col")
    nc.vector.memset(nshift_col, -SHIFT)

    # rounded fp8 tokens: t8 = x*s + b per channel block; cb0 on DVE, cb1 on
    # ScalarE run in parallel
    for sch in range(NSUB):
        asl = slice(sch * 512, (sch + 1) * 512)
        s_col0, b_col0 = gn_cols[0]
        nc.vector.tensor_scalar(
            out=t8[:, 0, asl], in0=t_cm[:, 0, asl], scalar1=s_col0,
            scalar2=b_col0, op0=OP.mult, op1=OP.add,
        )
        s_col1, b_col1 = gn_cols[1]
        nc.scalar.activation(
            out=t8[:, 1, asl], in_=t_cm[:, 1, asl], func=AF.Identity,
            bias=b_col1, scale=s_col1,
        )

    # ---- V3 storage with ones columns; q8 ----
    q8 = persist.tile([P, CB, N], FP8, tag="q8")
    v_aug = persist.tile([P, MB, C + 2], FP8, tag="v_aug")
    ones_c8 = persist.tile([P, 1], FP8, tag="ones_c8")
    nc.vector.memset(ones_c8, 1.0)
    nc.scalar.copy(out=v_aug[:, :, C : C + 2], in_=ones_c8.to_broadcast((P, MB, 2)))

    def q_chunk(ch):
        sl = slice(ch * NCHUNK, (ch + 1) * NCHUNK)
        pq = ps_pair.tile([P, 2, NCHUNK], F32, tag="ps_pair", name=f"pq_{ch}")
        for cb in range(CB):
            nc.tensor.matmul(
                pq[:, cb, :],
                lhsT=a8[:, :, cb * P : (cb + 1) * P],
                rhs=t8[:, :, sl],
                start=True,
                stop=False,
                perf_mode=DR,
            )
            nc.tensor.matmul(
                pq[:, cb, :],
                lhsT=a8l[:, :, cb * P : (cb + 1) * P],
                rhs=t8[:, :, sl],
                start=False,
                stop=True,
                perf_mode=DR,
            )
            nc.scalar.activation(
                out=q8[:, cb, sl], in_=pq[:, cb, :], func=AF.Identity,
                bias=w_col[:, cb : cb + 1], scale=1.0,
            )

    def v_pair_block(bp_, eng):
        vps = ps_pair.tile([P, 2, NCHUNK], F32, tag="ps_pair", name=f"vb_{bp_}")
        for h in range(2):
            nb = 2 * bp_ + h
            nc.tensor.matmul(
                vps[:, h, 0:C],
                lhsT=t8[:, :, nb * P : (nb + 1) * P],
                rhs=wvp8[:, :, :],
                start=True,
                stop=False,
                perf_mode=DR,
            )
            nc.tensor.matmul(
                vps[:, h, 0:C],
                lhsT=t8[:, :, nb * P : (nb + 1) * P],
                rhs=wvp8l[:, :, :],
                start=False,
                stop=False,
                perf_mode=DR,
            )
            # + ones x b' (residual+proj bias, recovered by the normalize fold)
            nc.tensor.matmul(
                vps[:, h, 0:C], lhsT=ones_bf, rhs=b_row, start=False, stop=True
            )
        if eng == "A":
            nc.scalar.copy(out=v_aug[:, 2 * bp_ : 2 * bp_ + 2, 0:C], in_=vps[:, :, 0:C])
        else:
            nc.vector.tensor_copy(
                out=v_aug[:, 2 * bp_ : 2 * bp_ + 2, 0:C], in_=vps[:, :, 0:C]
            )

    ex_const = None
    if exp_mode == "none":
        ex_const = persist.tile([P, 2, NCHUNK], FP8, tag="ex_const")
        nc.vector.memset(ex_const, 0.25)

    def produce_pair(j, bp_, with_v):
        if with_v:
            v_pair_block(bp_, "A" if bp_ % 2 else "D")
        jsl = slice((j % NJ) * NCHUNK, (j % NJ + 1) * NCHUNK)
        ss = ps_pair.tile([P, 2, NCHUNK], F32, tag="ps_pair", name=f"ss_{j}_{bp_}")
        for h in range(2):
            nc.tensor.matmul(
                ss[:, h, :],
                lhsT=t8[:, :, (2 * bp_ + h) * P : (2 * bp_ + h + 1) * P],
                rhs=q8[:, :, jsl],
                start=True,
                stop=True,
                perf_mode=DR,
            )
        if exp_mode == "none":
            return ex_const
        eng = PAIR_SCHED[bp_] if exp_mode == "mix" else ("A" if exp_mode == "act" else "D")
        ex = sexp.tile([P, 2, NCHUNK], FP8, tag="exp", name=f"ex_{j}_{bp_}")
        if eng == "A":
            nc.scalar.activation(
                out=ex, in_=ss, func=AF.Exp, scale=SCALE, bias=nshift_col
            )
        else:
            t32 = si32.tile([P, 2, NCHUNK], I32, tag="i32")
            nc.vector.tensor_scalar(
                out=t32, in0=ss, scalar1=EA, scalar2=EB, op0=OP.mult, op1=OP.add
            )
            nc.gpsimd.tensor_copy(out=ex, in_=t32.bitcast(F32))
        return ex

    def pv_mm(pv_ps, bp_, ex_pair):
        for jj in range(JJ):
            nc.tensor.matmul(
                pv_ps[jj],
                lhsT=ex_pair[:, :, jj * P : (jj + 1) * P],
                rhs=v_aug[:, 2 * bp_ : 2 * bp_ + 2, :],
                start=(bp_ == 0),
                stop=(bp_ == NPAIR - 1),
                perf_mode=DR,
            )

    def epilogue(j, pv_ps):
        jsl = slice((j % NJ) * NCHUNK, (j % NJ + 1) * NCHUNK)
        obs = [
            sout.tile([P, NCHUNK], F32, tag="out", name=f"ob_{j}_{co}")
            for co in range(CB)
        ]
        for jj in range(JJ):
            rec = stmp.tile([P, 1], F32, tag="rec")
            nc.vector.reciprocal(out=rec, in_=pv_ps[jj][:, C : C + 1])
            anm = stmp.tile([P, C], F32, tag="anm")
            nc.scalar.activation(
                out=anm, in_=pv_ps[jj][:, 0:C], func=AF.Copy, scale=rec
            )
            for co in range(CB):
                tp = ps_tp.tile([P, P], F32, tag="ps_tp")
                nc.tensor.transpose(tp, anm[:, co * P : (co + 1) * P], ident)
                nbase = (j % NJ) * NCHUNK + jj * P
                s_col, _ = gn_cols[co]
                nc.vector.scalar_tensor_tensor(
                    out=obs[co][:, jj * P : (jj + 1) * P],
                    in0=t_cm[:, co, nbase : nbase + P],
                    scalar=s_col,
                    in1=tp,
                    op0=OP.mult,
                    op1=OP.add,
                )
        for co in range(CB):
            nc.sync.dma_start(out=out_d[co * P : (co + 1) * P, jsl], in_=obs[co])

    def pv_tiles(j):
        return [
            ps_pv.tile([P, C + 2], F32, tag="ps_pv", name=f"pv_ps_{j}_{jj}")
            for jj in range(JJ)
        ]

    # ---- chunk 0, interleaved with V3 production ----
    LA = 3  # pair lookahead (<= ps_pair bufs - 1)
    q_chunk(0)
    pv_ps0 = pv_tiles(0)
    exs = {bp_: produce_pair(0, bp_, True) for bp_ in range(LA)}
    for bp_ in range(NPAIR):
        pv_mm(pv_ps0, bp_, exs.pop(bp_))
        if bp_ + LA < NPAIR:
            exs[bp_ + LA] = produce_pair(0, bp_ + LA, True)
    q_chunk(1)
    epilogue(0, pv_ps0)

    # ---- remaining chunks ----
    for j in range(1, NJ * att_reps):
        if 0 < j % NJ and j % NJ < NJ - 1 and j < NJ:
            q_chunk(j + 1)
        pv_ps = pv_tiles(j)
        exs = {bp_: produce_pair(j, bp_, False) for bp_ in range(LA)}
        for bp_ in range(NPAIR):
            pv_mm(pv_ps, bp_, exs.pop(bp_))
            if bp_ + LA < NPAIR:
                exs[bp_ + LA] = produce_pair(j, bp_ + LA, False)
        epilogue(j, pv_ps)


def kernel(x, gamma, beta, Wq, bq, Wk, bk, Wv, bv, Wp, bp):
    if "nc" not in _CACHE:
        _CACHE["nc"] = build_nc()
    nc = _CACHE["nc"]

    x = np.ascontiguousarray(np.asarray(x, dtype=np.float32)).reshape(B, C, N)
    common = {
        "gamma": np.asarray(gamma, np.float32),
        "beta": np.asarray(beta, np.float32),
        "Wq": np.asarray(Wq, np.float32),
        "bq": np.asarray(bq, np.float32),
        "Wk": np.asarray(Wk, np.float32),
        "bk": np.asarray(bk, np.float32),
        "Wv": np.asarray(Wv, np.float32),
        "bv": np.asarray(bv, np.float32),
        "Wp": np.asarray(Wp, np.float32),
        "bp": np.asarray(bp, np.float32),
    }
    in_maps = [{"x": x[b], **common} for b in range(B)]
    res = bass_utils.run_bass_kernel_spmd(nc, in_maps, core_ids=list(range(B)))
    out = np.stack([res.results[b]["out"] for b in range(B)])
    return out.reshape(B, C, H, W)


# revision 26
# speedup vs baseline: 1.7298x; 1.7298x over previous
"""Trainium2 Bass kernel for an AttentionBlock (GroupNorm + single-head
full N^2 attention + output projection + residual), data-parallel over
batch: 8 samples on 8 NeuronCores, no collectives.

Shapes (hardcoded): x [8, 256, 64, 64]; weights [256, 256]; biases [256].
Per core: one batch sample, x viewed as [C=256, N=4096] channel-major.

Per-core pipeline (fp8 DoubleRow matmuls: 2 k-tiles of 128 contracted per
instruction -> full C=256 contraction per matmul at double fp8 rate):
  1. GroupNorm (8 groups) in C-major layout: per-partition bn_stats,
     cross-partition group reduction via tiny constant matmuls, applied as
     per-partition scale/bias. Rounded tokens t8 (fp8e4) feed all matmuls;
     the fp32 residual out = x*s + proj' is applied in the epilogue via
     scalar_tensor_tensor, and the bias b' = b_gn + bp + Wp bv rides the
     V3 psum as a ones x b'-row outer product.
  2. Wq/Wk fold: scores = t A t^T with A = Wq^T Wk (computed on-chip,
     stored dual fp8 hi+lo). The bq bias becomes a +Wk^T bq column on the
     q2 projection (all other bias terms are per-query constants that
     cancel in softmax). The output projection folds into V:
     v3 = t (Wp Wv)^T (dual fp8), with appended ones columns so PV also
     produces softmax denominators.
  3. Attention over 512-query chunks, transposed: S^T = t8^T q8 with keys
     on partitions. Scores for a key-block PAIR live in one 2-bank psum
     tile so each exp instruction covers 1024 elems/partition. exp is
     split: ~9/16 pairs on ACT (native Exp -> fp8), ~7/16 via DVE
     Schraudolph (int32 = s*EA + EB == fp32 bits of exp(s*scale - SHIFT))
     with the int32 -> fp8 convert on GPSIMD (no PSUM port, so it only
     gets SBUF-side work). The global SHIFT keeps exp within fp8e4 range
     (max 240) and cancels in the normalization.
  4. PV accumulates [proj | colsum] in PSUM over 16 key-block pairs
     (DoubleRow). Epilogue per 128-query block: normalize by 1/colsum
     (ACT copy with per-partition scale), TensorE-transpose back to
     C-major, one scalar_tensor_tensor out = x*s + proj', DMA out.
"""

import numpy as np

import concourse.bacc as bacc
import concourse.mybir as mybir
import concourse.tile as tile
from concourse import bass_utils

F32 = mybir.dt.float32
F32R = mybir.dt.float32r
BF16 = mybir.dt.bfloat16
FP8 = mybir.dt.float8e4
I32 = mybir.dt.int32
AF = mybir.ActivationFunctionType
OP = mybir.AluOpType
DR = mybir.MatmulPerfMode.DoubleRow

B = 8
C = 256
H = 64
W = 64
N = H * W  # 4096 tokens
G = 8  # groups
GS = C // G  # 32 channels per group
P = 128
CB = C // P  # 2 channel blocks
EPS = 1e-5
NCHUNK = 256  # query chunk (matmul moving free dim)
NJ = N // NCHUNK  # 16
MB = N // P  # 32 key blocks
NPAIR = MB // 2  # 16 key-block pairs
JJ = NCHUNK // P  # 2 query sub-blocks per chunk
SCALE = C ** (-0.5)
SHIFT = 3.5  # exp(s*SCALE - SHIFT): keeps fp8e4 range, cancels in softmax

# Schraudolph exp-as-uint8-fp8-bits constants:
#   fp8e4_bits(exp(y)) ~= sat_u8_rne(y*8*log2e + 56), y = s*SCALE - SHIFT.
# DVE f32->u8 conversion saturates on HW (verified: neg -> 0, >255 -> 255,
# RNE), so one tensor_scalar per tile produces PV-ready fp8 weights.
LOG2E = 1.4426950408889634
EA8 = float(SCALE * 8.0 * LOG2E)
EB8 = float(56.0 - SHIFT * 8.0 * LOG2E)

# Engine split for the 16 exp pair-tiles per chunk: "A" = ACT native exp,
# "D" = DVE single-op uint8 Schraudolph.
PAIR_SCHED = ["A", "D"] * 7 + ["A", "A"]

_CACHE: dict = {}


def build_nc(att_reps=1, exp_mode="mix"):
    """exp_mode: "mix" (PAIR_SCHED), "act", "dve" (timing calibration),
    "none" (skip exp: PV reads a constant tile; output garbage)."""
    nc = bacc.Bacc(
        "TRN2",
        target_bir_lowering=False,
        debug=False,
        enable_asserts=False,
        num_devices=B,
    )

    x_d = nc.dram_tensor("x", [C, N], F32, kind="ExternalInput")
    gamma_d = nc.dram_tensor("gamma", [C], F32, kind="ExternalInput")
    beta_d = nc.dram_tensor("beta", [C], F32, kind="ExternalInput")
    w_d = {}
    b_d = {}
    for nm in ("q", "k", "v", "p"):
        w_d[nm] = nc.dram_tensor(f"W{nm}", [C, C], F32, kind="ExternalInput")
        b_d[nm] = nc.dram_tensor(f"b{nm}", [C], F32, kind="ExternalInput")
    out_d = nc.dram_tensor("out", [C, N], F32, kind="ExternalOutput")

    ident_d = nc.inline_tensor(np.eye(P, dtype=np.float32), name="ident")
    # Group-sum selector: [P, G/CB] with 1/GS entries -> group means directly.
    gsum_np = np.zeros((P, G // CB), np.float32)
    for p in range(P):
        gsum_np[p, p // GS] = 1.0 / GS
    gsum_d = nc.inline_tensor(gsum_np, name="gsum")
    # Group-broadcast selector: [G/CB, P] with 1s.
    gbc_np = np.zeros((G // CB, P), np.float32)
    for p in range(P):
        gbc_np[p // GS, p] = 1.0
    gbc_d = nc.inline_tensor(gbc_np, name="gbc")

    from contextlib import ExitStack

    with tile.TileContext(nc) as tc:
        with ExitStack() as ctx:
            _build_tile(
                ctx, tc, x_d, gamma_d, beta_d, w_d, b_d, out_d, ident_d, gsum_d,
                gbc_d, att_reps, exp_mode,
            )
    nc.compile()
    return nc


def _build_tile(ctx, tc, x_d, gamma_d, beta_d, w_d, b_d, out_d, ident_d, gsum_d, gbc_d, att_reps=1, exp_mode="mix"):
    nc = tc.nc

    persist = ctx.enter_context(tc.tile_pool(name="persist", bufs=1))
    staging = ctx.enter_context(tc.tile_pool(name="staging", bufs=4))
    sexp = ctx.enter_context(tc.tile_pool(name="sexp", bufs=6))
    si32 = ctx.enter_context(tc.tile_pool(name="si32", bufs=4))
    sout = ctx.enter_context(tc.tile_pool(name="sout", bufs=4))
    stmp = ctx.enter_context(tc.tile_pool(name="stmp", bufs=6))
    # ps_pair: [P, 2, NCHUNK] f32 tiles (1 psum bank each, bufs=4 -> 4 banks)
    ps_pair = ctx.enter_context(tc.tile_pool(name="ps_pair", bufs=4, space="PSUM"))
    # ps_pv: PV accumulators (1 bank each, JJ=2 alive per chunk)
    ps_pv = ctx.enter_context(tc.tile_pool(name="ps_pv", bufs=2, space="PSUM"))
    # ps_tp: epilogue transposes (decoupled from PV accumulator rotation)
    ps_tp = ctx.enter_context(tc.tile_pool(name="ps_tp", bufs=2, space="PSUM"))

    t_cm = persist.tile([P, CB, N], F32, tag="t_cm")  # raw x, C-major
    t8 = persist.tile([P, CB, N], FP8, tag="t8")  # groupnormed tokens, fp8
    NSUB = N // 512  # bn_stats free-dim limit

    # ---- x load first, spread over all 3 DMA-capable queues ----
    for s_ in range(NSUB):
        sl = slice(s_ * 512, (s_ + 1) * 512)
        if s_ < 4:
            nc.sync.dma_start(out=t_cm[:, 0, sl], in_=x_d[0:P, sl])
        else:
            nc.scalar.dma_start(out=t_cm[:, 0, sl], in_=x_d[0:P, sl])
    for s_ in range(NSUB):
        sl = slice(s_ * 512, (s_ + 1) * 512)
        eng = nc.sync if s_ < 2 else (nc.scalar if s_ < 4 else nc.gpsimd)
        eng.dma_start(out=t_cm[:, 1, sl], in_=x_d[P : 2 * P, sl])

    # small constants split across SP / ACT queues behind the x slices
    gsum = persist.tile([P, G // CB], F32, tag="gsum")
    nc.scalar.dma_start(out=gsum, in_=gsum_d[:, :])
    gbc = persist.tile([G // CB, P], F32, tag="gbc")
    nc.scalar.dma_start(out=gbc, in_=gbc_d[:, :])

    def col_tile(dram_vec, tag, eng):
        t = persist.tile([P, CB], F32, tag=tag)
        eng.dma_start(out=t, in_=dram_vec[:].rearrange("(b p) -> p b", p=P))
        return t

    gamma_col = col_tile(gamma_d, "gamma_col", nc.scalar)
    beta_col = col_tile(beta_d, "beta_col", nc.scalar)
    ident = persist.tile([P, P], F32, tag="ident")
    nc.sync.dma_start(out=ident, in_=ident_d[:, :])
    bq_col = col_tile(b_d["q"], "bq_col", nc.sync)
    bv_col = col_tile(b_d["v"], "bv_col", nc.sync)
    bp_col = col_tile(b_d["p"], "bp_col", nc.sync)

    # staged natural-layout weights [P, CB, C] (row r = b*128+p on partition p)
    # on the gpsimd queue (cheap triggers), right behind its x share
    w_stage = {}
    for nm in ("q", "k", "v", "p"):
        w_sb = staging.tile([P, CB, C], F32, tag="w_stage", name=f"w_sb_{nm}")
        nc.gpsimd.dma_start(
            out=w_sb, in_=w_d[nm][:, :].rearrange("(b p) i -> p b i", p=P)
        )
        w_stage[nm] = w_sb

    # ---- A = Wq^T Wk  [c, c'] as dual fp8 (hi + residual lo) ----
    a8 = persist.tile([P, CB, C], FP8, tag="a8")
    a8l = persist.tile([P, CB, C], FP8, tag="a8l")
    for cb in range(CB):
        aps = ps_pair.tile([P, C], F32, tag="ps_pair", name=f"aps_{cb}")
        for mb in range(CB):
            nc.tensor.matmul(
                aps,
                lhsT=w_stage["q"][:, mb, cb * P : (cb + 1) * P],
                rhs=w_stage["k"][:, mb, :],
                start=(mb == 0),
                stop=(mb == CB - 1),
            )
        nc.scalar.copy(out=a8[:, cb, :], in_=aps)
        nc.vector.tensor_tensor(
            out=a8l[:, cb, :], in0=aps, in1=a8[:, cb, :], op=OP.subtract
        )

    # ---- WpT via TensorE transposes: [P(m), CB(mb), C(c')] f32 ----
    wpT = persist.tile([P, CB, C], F32, tag="wpT")
    for b1 in range(CB):  # c' block (rows of Wp)
        for b2 in range(CB):  # m block
            tp = ps_pair.tile([P, P], F32, tag="ps_pair")
            nc.tensor.transpose(tp, w_stage["p"][:, b1, b2 * P : (b2 + 1) * P], ident)
            nc.scalar.copy(out=wpT[:, b2, b1 * P : (b1 + 1) * P], in_=tp)

    # ---- wvp8 = (Wp Wv)^T = Wv^T Wp^T  dual fp8 (hi + residual lo) ----
    wvp8 = persist.tile([P, CB, C], FP8, tag="wvp8")
    wvp8l = persist.tile([P, CB, C], FP8, tag="wvp8l")
    for ci_b in range(CB):
        pvp = ps_pair.tile([P, C], F32, tag="ps_pair", name=f"pvp_{ci_b}")
        for cm_b in range(CB):
            nc.tensor.matmul(
                pvp,
                lhsT=w_stage["v"][:, cm_b, ci_b * P : (ci_b + 1) * P],
                rhs=wpT[:, cm_b, :],
                start=(cm_b == 0),
                stop=(cm_b == CB - 1),
            )
        nc.scalar.copy(out=wvp8[:, ci_b, :], in_=pvp)
        nc.vector.tensor_tensor(
            out=wvp8l[:, ci_b, :], in0=pvp, in1=wvp8[:, ci_b, :], op=OP.subtract
        )

    # ---- w_col = Wk^T bq (q2 bias column), bv2_col = Wp bv ----
    w_col = persist.tile([P, CB], F32, tag="w_col")
    bv2_col = persist.tile([P, CB], F32, tag="bv2_col")
    for cb in range(CB):
        wps = ps_pair.tile([P, 1], F32, tag="ps_pair", name=f"wps_{cb}")
        for mb in range(CB):
            nc.tensor.matmul(
                wps,
                lhsT=w_stage["k"][:, mb, cb * P : (cb + 1) * P],
                rhs=bq_col[:, mb : mb + 1],
                start=(mb == 0),
                stop=(mb == CB - 1),
            )
        nc.vector.tensor_copy(out=w_col[:, cb : cb + 1], in_=wps)
        vps = ps_pair.tile([P, 1], F32, tag="ps_pair", name=f"vps_{cb}")
        for mb in range(CB):
            nc.tensor.matmul(
                vps,
                lhsT=wpT[:, mb, cb * P : (cb + 1) * P],
                rhs=bv_col[:, mb : mb + 1],
                start=(mb == 0),
                stop=(mb == CB - 1),
            )
        nc.vector.tensor_copy(out=bv2_col[:, cb : cb + 1], in_=vps)

    # ---- GroupNorm stats -> per-channel scale s_col, bias b_col ----
    gn_cols = []
    for cb in range(CB):
        xt = t_cm[:, cb, :]
        stats = stmp.tile([P, NSUB, 6], F32, tag="gn_stats")
        for s in range(NSUB):
            nc.vector.bn_stats(out=stats[:, s, :], in_=xt[:, s * 512 : (s + 1) * 512])
        mv = stmp.tile([P, 2], F32, tag="gn_mv")
        nc.vector.bn_aggr(out=mv, in_=stats)
        # stats2 = (mean_p, E[x^2]_p)
        stats2 = stmp.tile([P, 2], F32, tag="gn_stats2")
        nc.vector.tensor_copy(out=stats2[:, 0:1], in_=mv[:, 0:1])
        nc.vector.tensor_tensor(
            out=stats2[:, 1:2], in0=mv[:, 0:1], in1=mv[:, 0:1], op=OP.mult
        )
        nc.vector.tensor_add(out=stats2[:, 1:2], in0=stats2[:, 1:2], in1=mv[:, 1:2])
        # group reduce: [G/CB, 2] = gsum.T @ stats2  (means already /GS)
        gps = ps_pair.tile([G // CB, 2], F32, tag="ps_pair", name=f"gps_{cb}")
        nc.tensor.matmul(gps, lhsT=gsum, rhs=stats2, start=True, stop=True)
        # rstd_g = 1/sqrt(E2_g - mean_g^2 + eps)
        gsb = stmp.tile([G // CB, 2], F32, tag="gn_gsb")
        nc.vector.tensor_copy(out=gsb, in_=gps)
        gpack = stmp.tile([G // CB, 2], F32, tag="gn_gpack")
        nc.vector.tensor_copy(out=gpack[:, 0:1], in_=gsb[:, 0:1])
        gvar = stmp.tile([G // CB, 1], F32, tag="gn_gvar")
        nc.vector.tensor_tensor(
            out=gvar, in0=gsb[:, 0:1], in1=gsb[:, 0:1], op=OP.mult
        )
        nc.vector.tensor_tensor(
            out=gvar, in0=gsb[:, 1:2], in1=gvar, op=OP.subtract
        )
        eps_t = stmp.tile([G // CB, 1], F32, tag="gn_eps")
        nc.vector.memset(eps_t, EPS)
        nc.scalar.activation(out=gvar, in_=gvar, func=AF.Sqrt, bias=eps_t)
        nc.vector.reciprocal(out=gpack[:, 1:2], in_=gvar)
        # broadcast to channels: [P, 2] = gbc.T @ gpack
        bps = ps_pair.tile([P, 2], F32, tag="ps_pair", name=f"bps_{cb}")
        nc.tensor.matmul(bps, lhsT=gbc, rhs=gpack, start=True, stop=True)
        # s_col = rstd_c * gamma_c ; b_col = beta_c - mean_c * s_col
        s_col = stmp.tile([P, 1], F32, tag="gn_scol")
        nc.vector.tensor_tensor(
            out=s_col, in0=bps[:, 1:2], in1=gamma_col[:, cb : cb + 1], op=OP.mult
        )
        b_col = stmp.tile([P, 1], F32, tag="gn_bcol")
        nc.vector.tensor_tensor(out=b_col, in0=bps[:, 0:1], in1=s_col, op=OP.mult)
        nc.vector.tensor_tensor(
            out=b_col, in0=beta_col[:, cb : cb + 1], in1=b_col, op=OP.subtract
        )
        gn_cols.append((s_col, b_col))

    # ---- b' = b_gn + bp + Wp bv as a bf16 row [1, C] (for V3 psum fold) ----
    bsum_col = persist.tile([P, CB], F32, tag="bsum_col")
    for cb in range(CB):
        _, b_col = gn_cols[cb]
        nc.vector.tensor_add(
            out=bsum_col[:, cb : cb + 1], in0=b_col, in1=bp_col[:, cb : cb + 1]
        )
        nc.vector.tensor_add(
            out=bsum_col[:, cb : cb + 1],
            in0=bsum_col[:, cb : cb + 1],
            in1=bv2_col[:, cb : cb + 1],
        )
    bsum_bf = persist.tile([P, CB], BF16, tag="bsum_bf")
    nc.vector.tensor_copy(out=bsum_bf, in_=bsum_col)
    b_row = persist.tile([1, C], BF16, tag="b_row")
    for cb in range(CB):
        nc.sync.dma_start(
            out=b_row[0:1, cb * P : (cb + 1) * P], in_=bsum_bf[:, cb : cb + 1]
        )
    ones_bf = persist.tile([1, P], BF16, tag="ones_bf")
    nc.vector.memset(ones_bf, 1.0)
    nshift_col = persist.tile([P, 1], F32, tag="nshift_col")
    nc.vector.memset(nshift_col, -SHIFT)

    # rounded fp8 tokens: t8 = x*s + b per channel block; cb0 on DVE, cb1 on
    # ScalarE run in parallel
    for sch in range(NSUB):
        asl = slice(sch * 512, (sch + 1) * 512)
        s_col0, b_col0 = gn_cols[0]
        nc.vector.tensor_scalar(
            out=t8[:, 0, asl], in0=t_cm[:, 0, asl], scalar1=s_col0,
            scalar2=b_col0, op0=OP.mult, op1=OP.add,
        )
        s_col1, b_col1 = gn_cols[1]
        nc.scalar.activation(
            out=t8[:, 1, asl], in_=t_cm[:, 1, asl], func=AF.Identity,
            bias=b_col1, scale=s_col1,
        )

    # ---- V3 storage with ones columns; q8 ----
    q8 = persist.tile([P, CB, N], FP8, tag="q8")
    v_aug = persist.tile([P, MB, C + 2], FP8, tag="v_aug")
    ones_c8 = persist.tile([P, 1], FP8, tag="ones_c8")
    nc.vector.memset(ones_c8, 1.0)
    nc.scalar.copy(out=v_aug[:, :, C : C + 2], in_=ones_c8.to_broadcast((P, MB, 2)))

    def q_chunk(ch):
        sl = slice(ch * NCHUNK, (ch + 1) * NCHUNK)
        pq = ps_pair.tile([P, 2, NCHUNK], F32, tag="ps_pair", name=f"pq_{ch}")
        for cb in range(CB):
            nc.tensor.matmul(
                pq[:, cb, :],
                lhsT=a8[:, :, cb * P : (cb + 1) * P],
                rhs=t8[:, :, sl],
                start=True,
                stop=False,
                perf_mode=DR,
            )
            nc.tensor.matmul(
                pq[:, cb, :],
                lhsT=a8l[:, :, cb * P : (cb + 1) * P],
                rhs=t8[:, :, sl],
                start=False,
                stop=True,
                perf_mode=DR,
            )
            nc.scalar.activation(
                out=q8[:, cb, sl], in_=pq[:, cb, :], func=AF.Identity,
                bias=w_col[:, cb : cb + 1], scale=1.0,
            )

    def v_pair_block(bp_, eng):
        vps = ps_pair.tile([P, 2, NCHUNK], F32, tag="ps_pair", name=f"vb_{bp_}")
        for h in range(2):
            nb = 2 * bp_ + h
            nc.tensor.matmul(
                vps[:, h, 0:C],
                lhsT=t8[:, :, nb * P : (nb + 1) * P],
                rhs=wvp8[:, :, :],
                start=True,
                stop=False,
                perf_mode=DR,
            )
            nc.tensor.matmul(
                vps[:, h, 0:C],
                lhsT=t8[:, :, nb * P : (nb + 1) * P],
                rhs=wvp8l[:, :, :],
                start=False,
                stop=False,
                perf_mode=DR,
            )
            # + ones x b' (residual+proj bias, recovered by the normalize fold)
            nc.tensor.matmul(
                vps[:, h, 0:C], lhsT=ones_bf, rhs=b_row, start=False, stop=True
            )
        if eng == "A":
            nc.scalar.copy(out=v_aug[:, 2 * bp_ : 2 * bp_ + 2, 0:C], in_=vps[:, :, 0:C])
        else:
            nc.vector.tensor_copy(
                out=v_aug[:, 2 * bp_ : 2 * bp_ + 2, 0:C], in_=vps[:, :, 0:C]
            )

    ex_const = None
    if exp_mode == "none":
        ex_const = persist.tile([P, 2, NCHUNK], FP8, tag="ex_const")
        nc.vector.memset(ex_const, 0.25)

    def produce_pair(j, bp_, with_v):
        if with_v:
            v_pair_block(bp_, "A" if bp_ % 2 else "D")
        jsl = slice((j % NJ) * NCHUNK, (j % NJ + 1) * NCHUNK)
        ss = ps_pair.tile([P, 2, NCHUNK], F32, tag="ps_pair", name=f"ss_{j}_{bp_}")
        for h in range(2):
            nc.tensor.matmul(
                ss[:, h, :],
                lhsT=t8[:, :, (2 * bp_ + h) * P : (2 * bp_ + h + 1) * P],
                rhs=q8[:, :, jsl],
                start=True,
                stop=True,
                perf_mode=DR,
            )
        if exp_mode == "none":
            return ex_const
        eng = PAIR_SCHED[bp_] if exp_mode == "mix" else ("A" if exp_mode == "act" else "D")
        ex = sexp.tile([P, 2, NCHUNK], FP8, tag="exp", name=f"ex_{j}_{bp_}")
        if eng == "A":
            nc.scalar.activation(
                out=ex, in_=ss, func=AF.Exp, scale=SCALE, bias=nshift_col
            )
        else:
            nc.vector.tensor_scalar(
                out=ex.bitcast(mybir.dt.uint8), in0=ss,
                scalar1=EA8, scalar2=EB8, op0=OP.mult, op1=OP.add,
            )
        return ex

    def pv_mm(pv_ps, bp_, ex_pair):
        for jj in range(JJ):
            nc.tensor.matmul(
                pv_ps[jj],
                lhsT=ex_pair[:, :, jj * P : (jj + 1) * P],
                rhs=v_aug[:, 2 * bp_ : 2 * bp_ + 2, :],
                start=(bp_ == 0),
                stop=(bp_ == NPAIR - 1),
                perf_mode=DR,
            )

    def epilogue(j, pv_ps):
        jsl = slice((j % NJ) * NCHUNK, (j % NJ + 1) * NCHUNK)
        obs = [
            sout.tile([P, NCHUNK], F32, tag="out", name=f"ob_{j}_{co}")
            for co in range(CB)
        ]
        for jj in range(JJ):
            rec = stmp.tile([P, 1], F32, tag="rec")
            nc.vector.reciprocal(out=rec, in_=pv_ps[jj][:, C : C + 1])
            anm = stmp.tile([P, C], F32, tag="anm")
            nc.scalar.activation(
                out=anm, in_=pv_ps[jj][:, 0:C], func=AF.Copy, scale=rec
            )
            for co in range(CB):
                tp = ps_tp.tile([P, P], F32, tag="ps_tp")
                nc.tensor.transpose(tp, anm[:, co * P : (co + 1) * P], ident)
                nbase = (j % NJ) * NCHUNK + jj * P
                s_col, _ = gn_cols[co]
                nc.vector.scalar_tensor_tensor(
                    out=obs[co][:, jj * P : (jj + 1) * P],
                    in0=t_cm[:, co, nbase : nbase + P],
                    scalar=s_col,
                    in1=tp,
                    op0=OP.mult,
                    op1=OP.add,
                )
        for co in range(CB):
            nc.sync.dma_start(out=out_d[co * P : (co + 1) * P, jsl], in_=obs[co])

    def pv_tiles(j):
        return [
            ps_pv.tile([P, C + 2], F32, tag="ps_pv", name=f"pv_ps_{j}_{jj}")
            for jj in range(JJ)
        ]

    # ---- chunk 0, interleaved with V3 production ----
    LA = 3  # pair lookahead (<= ps_pair bufs - 1)
    q_chunk(0)
    pv_ps0 = pv_tiles(0)
    exs = {bp_: produce_pair(0, bp_, True) for bp_ in range(LA)}
    for bp_ in range(NPAIR):
        pv_mm(pv_ps0, bp_, exs.pop(bp_))
        if bp_ + LA < NPAIR:
            exs[bp_ + LA] = produce_pair(0, bp_ + LA, True)
    q_chunk(1)
    epilogue(0, pv_ps0)

    # ---- remaining chunks ----
    for j in range(1, NJ * att_reps):
        if 0 < j % NJ and j % NJ < NJ - 1 and j < NJ:
            q_chunk(j + 1)
        pv_ps = pv_tiles(j)
        exs = {bp_: produce_pair(j, bp_, False) for bp_ in range(LA)}
        for bp_ in range(NPAIR):
            pv_mm(pv_ps, bp_, exs.pop(bp_))
            if bp_ + LA < NPAIR:
                exs[bp_ + LA] = produce_pair(j, bp_ + LA, False)
        epilogue(j, pv_ps)


def kernel(x, gamma, beta, Wq, bq, Wk, bk, Wv, bv, Wp, bp):
    if "nc" not in _CACHE:
        _CACHE["nc"] = build_nc()
    nc = _CACHE["nc"]

    x = np.ascontiguousarray(np.asarray(x, dtype=np.float32)).reshape(B, C, N)
    common = {
        "gamma": np.asarray(gamma, np.float32),
        "beta": np.asarray(beta, np.float32),
        "Wq": np.asarray(Wq, np.float32),
        "bq": np.asarray(bq, np.float32),
        "Wk": np.asarray(Wk, np.float32),
        "bk": np.asarray(bk, np.float32),
        "Wv": np.asarray(Wv, np.float32),
        "bv": np.asarray(bv, np.float32),
        "Wp": np.asarray(Wp, np.float32),
        "bp": np.asarray(bp, np.float32),
    }
    in_maps = [{"x": x[b], **common} for b in range(B)]
    res = bass_utils.run_bass_kernel_spmd(nc, in_maps, core_ids=list(range(B)))
    out = np.stack([res.results[b]["out"] for b in range(B)])
    return out.reshape(B, C, H, W)


# revision 28
# speedup vs baseline: 1.7990x; 1.0400x over previous
"""Trainium2 Bass kernel for an AttentionBlock (GroupNorm + single-head
full N^2 attention + output projection + residual), data-parallel over
batch: 8 samples on 8 NeuronCores, no collectives.

Shapes (hardcoded): x [8, 256, 64, 64]; weights [256, 256]; biases [256].
Per core: one batch sample, x viewed as [C=256, N=4096] channel-major.

Per-core pipeline (fp8 DoubleRow matmuls: 2 k-tiles of 128 contracted per
instruction -> full C=256 contraction per matmul at double fp8 rate):
  1. GroupNorm (8 groups) in C-major layout: per-partition bn_stats,
     cross-partition group reduction via tiny constant matmuls, applied as
     per-partition scale/bias. Rounded tokens t8 (fp8e4) feed all matmuls;
     the fp32 residual out = x*s + proj' is applied in the epilogue via
     scalar_tensor_tensor, and the bias b' = b_gn + bp + Wp bv rides the
     V3 psum as a ones x b'-row outer product.
  2. Wq/Wk fold: scores = t A t^T with A = Wq^T Wk (computed on-chip,
     stored dual fp8 hi+lo). The bq bias becomes a +Wk^T bq column on the
     q2 projection (all other bias terms are per-query constants that
     cancel in softmax). The output projection folds into V:
     v3 = t (Wp Wv)^T (dual fp8), with appended ones columns so PV also
     produces softmax denominators.
  3. Attention over 256-query chunks, transposed: S^T = t8^T q8 with keys
     on partitions. Scores for a key-block PAIR live in one 1-bank psum
     tile (4 tiles in flight -> depth-4 exp pipeline). exp is split 9:7
     per chunk between ACT (native Exp -> fp8, HW-fast) and DVE via a
     single-op Schraudolph: fp8e4 bits of exp(s*SCALE - SHIFT) equal
     sat_u8_rne(s*EA8 + EB8), and the DVE f32->u8 convert saturates on
     HW (verified), so one tensor_scalar yields PV-ready fp8 weights.
     The global SHIFT keeps exp within fp8e4 range (max 240) and cancels
     in the normalization.
  4. PV accumulates [proj | colsum] in PSUM over 16 key-block pairs
     (DoubleRow). Epilogue per 128-query block: normalize by 1/colsum
     (ACT copy with per-partition scale), TensorE-transpose back to
     C-major, one scalar_tensor_tensor out = x*s + proj', DMA out.
"""

import numpy as np

import concourse.bacc as bacc
import concourse.mybir as mybir
import concourse.tile as tile
from concourse import bass_utils

F32 = mybir.dt.float32
F32R = mybir.dt.float32r
BF16 = mybir.dt.bfloat16
FP8 = mybir.dt.float8e4
I32 = mybir.dt.int32
AF = mybir.ActivationFunctionType
OP = mybir.AluOpType
DR = mybir.MatmulPerfMode.DoubleRow

B = 8
C = 256
H = 64
W = 64
N = H * W  # 4096 tokens
G = 8  # groups
GS = C // G  # 32 channels per group
P = 128
CB = C // P  # 2 channel blocks
EPS = 1e-5
NCHUNK = 256  # query chunk (matmul moving free dim)
NJ = N // NCHUNK  # 16
MB = N // P  # 32 key blocks
NPAIR = MB // 2  # 16 key-block pairs
JJ = NCHUNK // P  # 2 query sub-blocks per chunk
SCALE = C ** (-0.5)
SHIFT = 3.5  # exp(s*SCALE - SHIFT): keeps fp8e4 range, cancels in softmax

# Schraudolph exp-as-uint8-fp8-bits constants:
#   fp8e4_bits(exp(y)) ~= sat_u8_rne(y*8*log2e + 56), y = s*SCALE - SHIFT.
# DVE f32->u8 conversion saturates on HW (verified: neg -> 0, >255 -> 255,
# RNE), so one tensor_scalar per tile produces PV-ready fp8 weights.
LOG2E = 1.4426950408889634
EA8 = float(SCALE * 8.0 * LOG2E)
EB8 = float(56.0 - SHIFT * 8.0 * LOG2E)

# Engine split for the 16 exp pair-tiles per chunk: "A" = ACT native exp,
# "D" = DVE single-op uint8 Schraudolph.
PAIR_SCHED = ["A", "D"] * 7 + ["A", "A"]

_CACHE: dict = {}


def build_nc(att_reps=1, exp_mode="mix"):
    """exp_mode: "mix" (PAIR_SCHED), "act", "dve" (timing calibration),
    "none" (skip exp: PV reads a constant tile; output garbage)."""
    nc = bacc.Bacc(
        "TRN2",
        target_bir_lowering=False,
        debug=False,
        enable_asserts=False,
        num_devices=B,
    )

    x_d = nc.dram_tensor("x", [C, N], F32, kind="ExternalInput")
    gamma_d = nc.dram_tensor("gamma", [C], F32, kind="ExternalInput")
    beta_d = nc.dram_tensor("beta", [C], F32, kind="ExternalInput")
    w_d = {}
    b_d = {}
    for nm in ("q", "k", "v", "p"):
        w_d[nm] = nc.dram_tensor(f"W{nm}", [C, C], F32, kind="ExternalInput")
        b_d[nm] = nc.dram_tensor(f"b{nm}", [C], F32, kind="ExternalInput")
    out_d = nc.dram_tensor("out", [C, N], F32, kind="ExternalOutput")

    ident_d = nc.inline_tensor(np.eye(P, dtype=np.float32), name="ident")
    # Group-sum selector: [P, G/CB] with 1/GS entries -> group means directly.
    gsum_np = np.zeros((P, G // CB), np.float32)
    for p in range(P):
        gsum_np[p, p // GS] = 1.0 / GS
    gsum_d = nc.inline_tensor(gsum_np, name="gsum")
    # Group-broadcast selector: [G/CB, P] with 1s.
    gbc_np = np.zeros((G // CB, P), np.float32)
    for p in range(P):
        gbc_np[p // GS, p] = 1.0
    gbc_d = nc.inline_tensor(gbc_np, name="gbc")

    from contextlib import ExitStack

    with tile.TileContext(nc) as tc:
        with ExitStack() as ctx:
            _build_tile(
                ctx, tc, x_d, gamma_d, beta_d, w_d, b_d, out_d, ident_d, gsum_d,
                gbc_d, att_reps, exp_mode,
            )
    nc.compile()
    return nc


def _build_tile(ctx, tc, x_d, gamma_d, beta_d, w_d, b_d, out_d, ident_d, gsum_d, gbc_d, att_reps=1, exp_mode="mix"):
    nc = tc.nc

    persist = ctx.enter_context(tc.tile_pool(name="persist", bufs=1))
    staging = ctx.enter_context(tc.tile_pool(name="staging", bufs=4))
    sexp = ctx.enter_context(tc.tile_pool(name="sexp", bufs=6))
    si32 = ctx.enter_context(tc.tile_pool(name="si32", bufs=4))
    sout = ctx.enter_context(tc.tile_pool(name="sout", bufs=4))
    stmp = ctx.enter_context(tc.tile_pool(name="stmp", bufs=6))
    # ps_pair: [P, 2, NCHUNK] f32 tiles (1 psum bank each, bufs=4 -> 4 banks)
    ps_pair = ctx.enter_context(tc.tile_pool(name="ps_pair", bufs=4, space="PSUM"))
    # ps_pv: PV accumulators (1 bank each, JJ=2 alive per chunk)
    ps_pv = ctx.enter_context(tc.tile_pool(name="ps_pv", bufs=2, space="PSUM"))
    # ps_tp: epilogue transposes (decoupled from PV accumulator rotation)
    ps_tp = ctx.enter_context(tc.tile_pool(name="ps_tp", bufs=2, space="PSUM"))

    t_cm = persist.tile([P, CB, N], F32, tag="t_cm")  # raw x, C-major
    t8 = persist.tile([P, CB, N], FP8, tag="t8")  # groupnormed tokens, fp8
    NSUB = N // 512  # bn_stats free-dim limit

    # ---- x load in 1024-col slices, spread over the 3 DMA-capable queues;
    # GN constants (gsum/gbc) lead the scalar queue, weights ride gpsimd/sync
    gsum = persist.tile([P, G // CB], F32, tag="gsum")
    nc.scalar.dma_start(out=gsum, in_=gsum_d[:, :])
    gbc = persist.tile([G // CB, P], F32, tag="gbc")
    nc.scalar.dma_start(out=gbc, in_=gbc_d[:, :])

    XS = 1024
    x_q = [nc.sync, nc.sync, nc.sync, nc.scalar, nc.scalar, nc.scalar,
           nc.gpsimd, nc.gpsimd]
    for s_ in range(8):
        cb, i_ = divmod(s_, 4)
        sl = slice(i_ * XS, (i_ + 1) * XS)
        x_q[s_].dma_start(out=t_cm[:, cb, sl], in_=x_d[cb * P : (cb + 1) * P, sl])

    # staged natural-layout weights [P, CB, C] (row r = b*128+p on partition p)
    w_stage = {}
    for nm, eng in (("q", nc.gpsimd), ("k", nc.gpsimd), ("v", nc.sync), ("p", nc.sync)):
        w_sb = staging.tile([P, CB, C], F32, tag="w_stage", name=f"w_sb_{nm}")
        eng.dma_start(out=w_sb, in_=w_d[nm][:, :].rearrange("(b p) i -> p b i", p=P))
        w_stage[nm] = w_sb

    def col_tile(dram_vec, tag, eng):
        t = persist.tile([P, CB], F32, tag=tag)
        eng.dma_start(out=t, in_=dram_vec[:].rearrange("(b p) -> p b", p=P))
        return t

    gamma_col = col_tile(gamma_d, "gamma_col", nc.scalar)
    beta_col = col_tile(beta_d, "beta_col", nc.scalar)
    ident = persist.tile([P, P], F32, tag="ident")
    nc.gpsimd.dma_start(out=ident, in_=ident_d[:, :])
    bq_col = col_tile(b_d["q"], "bq_col", nc.gpsimd)
    bv_col = col_tile(b_d["v"], "bv_col", nc.gpsimd)
    bp_col = col_tile(b_d["p"], "bp_col", nc.gpsimd)

    # ---- A = Wq^T Wk  [c, c'] as dual fp8 (hi + residual lo) ----
    a8 = persist.tile([P, CB, C], FP8, tag="a8")
    a8l = persist.tile([P, CB, C], FP8, tag="a8l")
    for cb in range(CB):
        aps = ps_pair.tile([P, C], F32, tag="ps_pair", name=f"aps_{cb}")
        for mb in range(CB):
            nc.tensor.matmul(
                aps,
                lhsT=w_stage["q"][:, mb, cb * P : (cb + 1) * P],
                rhs=w_stage["k"][:, mb, :],
                start=(mb == 0),
                stop=(mb == CB - 1),
            )
        nc.scalar.copy(out=a8[:, cb, :], in_=aps)
        nc.vector.tensor_tensor(
            out=a8l[:, cb, :], in0=aps, in1=a8[:, cb, :], op=OP.subtract
        )

    # ---- WpT via TensorE transposes: [P(m), CB(mb), C(c')] f32 ----
    wpT = persist.tile([P, CB, C], F32, tag="wpT")
    for b1 in range(CB):  # c' block (rows of Wp)
        for b2 in range(CB):  # m block
            tp = ps_pair.tile([P, P], F32, tag="ps_pair")
            nc.tensor.transpose(tp, w_stage["p"][:, b1, b2 * P : (b2 + 1) * P], ident)
            nc.scalar.copy(out=wpT[:, b2, b1 * P : (b1 + 1) * P], in_=tp)

    # ---- wvp8 = (Wp Wv)^T = Wv^T Wp^T  dual fp8 (hi + residual lo) ----
    wvp8 = persist.tile([P, CB, C], FP8, tag="wvp8")
    wvp8l = persist.tile([P, CB, C], FP8, tag="wvp8l")
    for ci_b in range(CB):
        pvp = ps_pair.tile([P, C], F32, tag="ps_pair", name=f"pvp_{ci_b}")
        for cm_b in range(CB):
            nc.tensor.matmul(
                pvp,
                lhsT=w_stage["v"][:, cm_b, ci_b * P : (ci_b + 1) * P],
                rhs=wpT[:, cm_b, :],
                start=(cm_b == 0),
                stop=(cm_b == CB - 1),
            )
        nc.scalar.copy(out=wvp8[:, ci_b, :], in_=pvp)
        nc.vector.tensor_tensor(
            out=wvp8l[:, ci_b, :], in0=pvp, in1=wvp8[:, ci_b, :], op=OP.subtract
        )

    # ---- w_col = Wk^T bq (q2 bias column), bv2_col = Wp bv ----
    w_col = persist.tile([P, CB], F32, tag="w_col")
    bv2_col = persist.tile([P, CB], F32, tag="bv2_col")
    for cb in range(CB):
        wps = ps_pair.tile([P, 1], F32, tag="ps_pair", name=f"wps_{cb}")
        for mb in range(CB):
            nc.tensor.matmul(
                wps,
                lhsT=w_stage["k"][:, mb, cb * P : (cb + 1) * P],
                rhs=bq_col[:, mb : mb + 1],
                start=(mb == 0),
                stop=(mb == CB - 1),
            )
        nc.vector.tensor_copy(out=w_col[:, cb : cb + 1], in_=wps)
        vps = ps_pair.tile([P, 1], F32, tag="ps_pair", name=f"vps_{cb}")
        for mb in range(CB):
            nc.tensor.matmul(
                vps,
                lhsT=wpT[:, mb, cb * P : (cb + 1) * P],
                rhs=bv_col[:, mb : mb + 1],
                start=(mb == 0),
                stop=(mb == CB - 1),
            )
        nc.vector.tensor_copy(out=bv2_col[:, cb : cb + 1], in_=vps)

    # ---- GroupNorm stats -> per-channel scale s_col, bias b_col ----
    gn_cols = []
    for cb in range(CB):
        xt = t_cm[:, cb, :]
        stats = stmp.tile([P, NSUB, 6], F32, tag="gn_stats")
        for s in range(NSUB):
            nc.vector.bn_stats(out=stats[:, s, :], in_=xt[:, s * 512 : (s + 1) * 512])
        mv = stmp.tile([P, 2], F32, tag="gn_mv")
        nc.vector.bn_aggr(out=mv, in_=stats)
        # stats2 = (mean_p, E[x^2]_p)
        stats2 = stmp.tile([P, 2], F32, tag="gn_stats2")
        nc.vector.tensor_copy(out=stats2[:, 0:1], in_=mv[:, 0:1])
        nc.vector.tensor_tensor(
            out=stats2[:, 1:2], in0=mv[:, 0:1], in1=mv[:, 0:1], op=OP.mult
        )
        nc.vector.tensor_add(out=stats2[:, 1:2], in0=stats2[:, 1:2], in1=mv[:, 1:2])
        # group reduce: [G/CB, 2] = gsum.T @ stats2  (means already /GS)
        gps = ps_pair.tile([G // CB, 2], F32, tag="ps_pair", name=f"gps_{cb}")
        nc.tensor.matmul(gps, lhsT=gsum, rhs=stats2, start=True, stop=True)
        # rstd_g = 1/sqrt(E2_g - mean_g^2 + eps)
        gsb = stmp.tile([G // CB, 2], F32, tag="gn_gsb")
        nc.vector.tensor_copy(out=gsb, in_=gps)
        gpack = stmp.tile([G // CB, 2], F32, tag="gn_gpack")
        nc.vector.tensor_copy(out=gpack[:, 0:1], in_=gsb[:, 0:1])
        gvar = stmp.tile([G // CB, 1], F32, tag="gn_gvar")
        nc.vector.tensor_tensor(
            out=gvar, in0=gsb[:, 0:1], in1=gsb[:, 0:1], op=OP.mult
        )
        nc.vector.tensor_tensor(
            out=gvar, in0=gsb[:, 1:2], in1=gvar, op=OP.subtract
        )
        eps_t = stmp.tile([G // CB, 1], F32, tag="gn_eps")
        nc.vector.memset(eps_t, EPS)
        nc.scalar.activation(out=gvar, in_=gvar, func=AF.Sqrt, bias=eps_t)
        nc.vector.reciprocal(out=gpack[:, 1:2], in_=gvar)
        # broadcast to channels: [P, 2] = gbc.T @ gpack
        bps = ps_pair.tile([P, 2], F32, tag="ps_pair", name=f"bps_{cb}")
        nc.tensor.matmul(bps, lhsT=gbc, rhs=gpack, start=True, stop=True)
        # s_col = rstd_c * gamma_c ; b_col = beta_c - mean_c * s_col
        s_col = stmp.tile([P, 1], F32, tag="gn_scol")
        nc.vector.tensor_tensor(
            out=s_col, in0=bps[:, 1:2], in1=gamma_col[:, cb : cb + 1], op=OP.mult
        )
        b_col = stmp.tile([P, 1], F32, tag="gn_bcol")
        nc.vector.tensor_tensor(out=b_col, in0=bps[:, 0:1], in1=s_col, op=OP.mult)
        nc.vector.tensor_tensor(
            out=b_col, in0=beta_col[:, cb : cb + 1], in1=b_col, op=OP.subtract
        )
        gn_cols.append((s_col, b_col))

    # ---- b' = b_gn + bp + Wp bv as a bf16 row [1, C] (for V3 psum fold) ----
    bsum_col = persist.tile([P, CB], F32, tag="bsum_col")
    for cb in range(CB):
        _, b_col = gn_cols[cb]
        nc.vector.tensor_add(
            out=bsum_col[:, cb : cb + 1], in0=b_col, in1=bp_col[:, cb : cb + 1]
        )
        nc.vector.tensor_add(
            out=bsum_col[:, cb : cb + 1],
            in0=bsum_col[:, cb : cb + 1],
            in1=bv2_col[:, cb : cb + 1],
        )
    bsum_bf = persist.tile([P, CB], BF16, tag="bsum_bf")
    nc.vector.tensor_copy(out=bsum_bf, in_=bsum_col)
    b_row = persist.tile([1, C], BF16, tag="b_row")
    for cb in range(CB):
        nc.sync.dma_start(
            out=b_row[0:1, cb * P : (cb + 1) * P], in_=bsum_bf[:, cb : cb + 1]
        )
    ones_bf = persist.tile([1, P], BF16, tag="ones_bf")
    nc.vector.memset(ones_bf, 1.0)
    nshift_col = persist.tile([P, 1], F32, tag="nshift_col")
    nc.vector.memset(nshift_col, -SHIFT)

    # rounded fp8 tokens: t8 = x*s + b per channel block; cb0 on DVE, cb1 on
    # ScalarE run in parallel
    for sch in range(NSUB):
        asl = slice(sch * 512, (sch + 1) * 512)
        s_col0, b_col0 = gn_cols[0]
        nc.vector.tensor_scalar(
            out=t8[:, 0, asl], in0=t_cm[:, 0, asl], scalar1=s_col0,
            scalar2=b_col0, op0=OP.mult, op1=OP.add,
        )
        s_col1, b_col1 = gn_cols[1]
        nc.scalar.activation(
            out=t8[:, 1, asl], in_=t_cm[:, 1, asl], func=AF.Identity,
            bias=b_col1, scale=s_col1,
        )

    # ---- V3 storage with ones columns; q8 ----
    q8 = persist.tile([P, CB, N], FP8, tag="q8")
    v_aug = persist.tile([P, MB, C + 2], FP8, tag="v_aug")
    ones_c8 = persist.tile([P, 1], FP8, tag="ones_c8")
    nc.vector.memset(ones_c8, 1.0)
    nc.scalar.copy(out=v_aug[:, :, C : C + 2], in_=ones_c8.to_broadcast((P, MB, 2)))

    def q_chunk(ch):
        sl = slice(ch * NCHUNK, (ch + 1) * NCHUNK)
        pq = ps_pair.tile([P, 2, NCHUNK], F32, tag="ps_pair", name=f"pq_{ch}")
        for cb in range(CB):
            nc.tensor.matmul(
                pq[:, cb, :],
                lhsT=a8[:, :, cb * P : (cb + 1) * P],
                rhs=t8[:, :, sl],
                start=True,
                stop=False,
                perf_mode=DR,
            )
            nc.tensor.matmul(
                pq[:, cb, :],
                lhsT=a8l[:, :, cb * P : (cb + 1) * P],
                rhs=t8[:, :, sl],
                start=False,
                stop=True,
                perf_mode=DR,
            )
            nc.scalar.activation(
                out=q8[:, cb, sl], in_=pq[:, cb, :], func=AF.Identity,
                bias=w_col[:, cb : cb + 1], scale=1.0,
            )

    def v_pair_block(bp_, eng):
        vps = ps_pair.tile([P, 2, NCHUNK], F32, tag="ps_pair", name=f"vb_{bp_}")
        for h in range(2):
            nb = 2 * bp_ + h
            nc.tensor.matmul(
                vps[:, h, 0:C],
                lhsT=t8[:, :, nb * P : (nb + 1) * P],
                rhs=wvp8[:, :, :],
                start=True,
                stop=False,
                perf_mode=DR,
            )
            nc.tensor.matmul(
                vps[:, h, 0:C],
                lhsT=t8[:, :, nb * P : (nb + 1) * P],
                rhs=wvp8l[:, :, :],
                start=False,
                stop=False,
                perf_mode=DR,
            )
            # + ones x b' (residual+proj bias, recovered by the normalize fold)
            nc.tensor.matmul(
                vps[:, h, 0:C], lhsT=ones_bf, rhs=b_row, start=False, stop=True
            )
        if eng == "A":
            nc.scalar.copy(out=v_aug[:, 2 * bp_ : 2 * bp_ + 2, 0:C], in_=vps[:, :, 0:C])
        else:
            nc.vector.tensor_copy(
                out=v_aug[:, 2 * bp_ : 2 * bp_ + 2, 0:C], in_=vps[:, :, 0:C]
            )

    ex_const = None
    if exp_mode == "none":
        ex_const = persist.tile([P, 2, NCHUNK], FP8, tag="ex_const")
        nc.vector.memset(ex_const, 0.25)

    def produce_pair(j, bp_, with_v):
        if with_v:
            v_pair_block(bp_, "A" if bp_ % 2 else "D")
        jsl = slice((j % NJ) * NCHUNK, (j % NJ + 1) * NCHUNK)
        ss = ps_pair.tile([P, 2, NCHUNK], F32, tag="ps_pair", name=f"ss_{j}_{bp_}")
        for h in range(2):
            nc.tensor.matmul(
                ss[:, h, :],
                lhsT=t8[:, :, (2 * bp_ + h) * P : (2 * bp_ + h + 1) * P],
                rhs=q8[:, :, jsl],
                start=True,
                stop=True,
                perf_mode=DR,
            )
        if exp_mode == "none":
            return ex_const
        eng = PAIR_SCHED[bp_] if exp_mode == "mix" else ("A" if exp_mode == "act" else "D")
        ex = sexp.tile([P, 2, NCHUNK], FP8, tag="exp", name=f"ex_{j}_{bp_}")
        if eng == "A":
            nc.scalar.activation(
                out=ex, in_=ss, func=AF.Exp, scale=SCALE, bias=nshift_col
            )
        else:
            nc.vector.tensor_scalar(
                out=ex.bitcast(mybir.dt.uint8), in0=ss,
                scalar1=EA8, scalar2=EB8, op0=OP.mult, op1=OP.add,
            )
        return ex

    def pv_mm(pv_ps, bp_, ex_pair):
        for jj in range(JJ):
            nc.tensor.matmul(
                pv_ps[jj],
                lhsT=ex_pair[:, :, jj * P : (jj + 1) * P],
                rhs=v_aug[:, 2 * bp_ : 2 * bp_ + 2, :],
                start=(bp_ == 0),
                stop=(bp_ == NPAIR - 1),
                perf_mode=DR,
            )

    def epilogue(j, pv_ps):
        jsl = slice((j % NJ) * NCHUNK, (j % NJ + 1) * NCHUNK)
        obs = [
            sout.tile([P, NCHUNK], F32, tag="out", name=f"ob_{j}_{co}")
            for co in range(CB)
        ]
        for jj in range(JJ):
            rec = stmp.tile([P, 1], F32, tag="rec")
            nc.vector.reciprocal(out=rec, in_=pv_ps[jj][:, C : C + 1])
            anm = stmp.tile([P, C], F32, tag="anm")
            nc.scalar.activation(
                out=anm, in_=pv_ps[jj][:, 0:C], func=AF.Copy, scale=rec
            )
            for co in range(CB):
                tp = ps_tp.tile([P, P], F32, tag="ps_tp")
                nc.tensor.transpose(tp, anm[:, co * P : (co + 1) * P], ident)
                nbase = (j % NJ) * NCHUNK + jj * P
                s_col, _ = gn_cols[co]
                nc.vector.scalar_tensor_tensor(
                    out=obs[co][:, jj * P : (jj + 1) * P],
                    in0=t_cm[:, co, nbase : nbase + P],
                    scalar=s_col,
                    in1=tp,
                    op0=OP.mult,
                    op1=OP.add,
                )
        for co in range(CB):
            nc.sync.dma_start(out=out_d[co * P : (co + 1) * P, jsl], in_=obs[co])

    def pv_tiles(j):
        return [
            ps_pv.tile([P, C + 2], F32, tag="ps_pv", name=f"pv_ps_{j}_{jj}")
            for jj in range(JJ)
        ]

    # ---- chunk 0, interleaved with V3 production ----
    LA = 3  # pair lookahead (<= ps_pair bufs - 1)
    q_chunk(0)
    pv_ps0 = pv_tiles(0)
    exs = {bp_: produce_pair(0, bp_, True) for bp_ in range(LA)}
    for bp_ in range(NPAIR):
        pv_mm(pv_ps0, bp_, exs.pop(bp_))
        if bp_ + LA < NPAIR:
            exs[bp_ + LA] = produce_pair(0, bp_ + LA, True)
    q_chunk(1)
    epilogue(0, pv_ps0)

    # ---- remaining chunks ----
    for j in range(1, NJ * att_reps):
        if 0 < j % NJ and j % NJ < NJ - 1 and j < NJ:
            q_chunk(j + 1)
        pv_ps = pv_tiles(j)
        exs = {bp_: produce_pair(j, bp_, False) for bp_ in range(LA)}
        for bp_ in range(NPAIR):
            pv_mm(pv_ps, bp_, exs.pop(bp_))
            if bp_ + LA < NPAIR:
                exs[bp_ + LA] = produce_pair(j, bp_ + LA, False)
        epilogue(j, pv_ps)


def kernel(x, gamma, beta, Wq, bq, Wk, bk, Wv, bv, Wp, bp):
    if "nc" not in _CACHE:
        _CACHE["nc"] = build_nc()
    nc = _CACHE["nc"]

    x = np.ascontiguousarray(np.asarray(x, dtype=np.float32)).reshape(B, C, N)
    common = {
        "gamma": np.asarray(gamma, np.float32),
        "beta": np.asarray(beta, np.float32),
        "Wq": np.asarray(Wq, np.float32),
        "bq": np.asarray(bq, np.float32),
        "Wk": np.asarray(Wk, np.float32),
        "bk": np.asarray(bk, np.float32),
        "Wv": np.asarray(Wv, np.float32),
        "bv": np.asarray(bv, np.float32),
        "Wp": np.asarray(Wp, np.float32),
        "bp": np.asarray(bp, np.float32),
    }
    in_maps = [{"x": x[b], **common} for b in range(B)]
    res = bass_utils.run_bass_kernel_spmd(nc, in_maps, core_ids=list(range(B)))
    out = np.stack([res.results[b]["out"] for b in range(B)])
    return out.reshape(B, C, H, W)
